# revision 1
# baseline (speedup 1.0000x reference)
"""APR conv (gather + per-particle stencil select) + GroupNorm + ReLU on 8 trn2 cores.

Sharding: particles (last axis N) split across 8 cores; the gather source
x^T [N, 16] f32 is replicated on every core; stencil weights / GN params
replicated; GroupNorm reduction over N becomes an AllReduce of per-group
(sum, sumsq).

Per-core pipeline (phase 1, per 2048-particle supertile):
  - one strided DMA loads neighbor_idx block [128, 16, 27] i32
  - 4 indirect DMA gathers (k-chunks of 8,8,8,3) pull 64B feature rows from
    x^T in DRAM -> SBUF [128p, nsub, kc*16]
  - PE transposes [128,128] blocks so (k, c) lands on partitions
  - PSUM-accumulated matmuls vs packed weights [(k c), (s o)] -> acc [96, 512]
  - DVE copy_predicated selects the stencil per particle -> sel [32, 512]
  - per-tile sum / sumsq accumulate for GroupNorm; sel written to DRAM scratch
Phase 2: per-core (sum, sumsq) [32, 2] -> AllReduce(add) over 8 cores ->
  scale/bias [32, 1] per channel.
Phase 3: read scratch back, one ACT op per tile: Relu(y * scale + bias).
"""

import math
from contextlib import ExitStack

import numpy as np

import concourse.bacc as bacc
import concourse.bass as bass
import concourse.tile as tile
from concourse import mybir
from concourse.bass_utils import run_bass_kernel_spmd

N_CORES = 8
C_IN = 16
C_OUT = 32
NS = 3  # stencils
K = 27
NGROUPS = 8
EPS = 1e-5
P = 128

K_CHUNKS = [(0, 8), (8, 8), (16, 8), (24, 3)]
NSO = NS * C_OUT  # 96

f32 = mybir.dt.float32
f32r = mybir.dt.float32r
i32 = mybir.dt.int32


def geometry(n_core: int, nsub: int):
    """Supertile list [(base, nsub_s)] covering n_core, padded per tile."""
    tiles = []
    base = 0
    while base < n_core:
        rem = n_core - base
        ns = nsub if rem >= P * nsub else (rem + P - 1) // P
        tiles.append((base, ns))
        base += ns * P
    return tiles, base


def build_nc(n_total: int, nsub: int = 16, mm: int = 512, use_f32r: bool = False, debug: bool = False):
    """Build the per-core Bass program (SPMD; same program on all cores)."""
    assert n_total % N_CORES == 0
    n_core = n_total // N_CORES
    SUP = P * nsub  # particles per (full) supertile
    assert SUP % mm == 0 and mm % P == 0
    sts, n_pad = geometry(n_core, nsub)
    n_sup = len(sts)
    NT = sum((ns * P + mm - 1) // mm for _, ns in sts)  # stat slots

    nc = bacc.Bacc(
        "TRN2",
        target_bir_lowering=False,
        debug=False,
        num_devices=N_CORES,
    )

    # ---- I/O ----
    xt_h = nc.dram_tensor("xt", [n_total, C_IN], f32, kind="ExternalInput")
    nidx_h = nc.dram_tensor("nidx", [n_pad, K], i32, kind="ExternalInput")
    ld_h = nc.dram_tensor("ld", [n_pad], f32, kind="ExternalInput")
    w_h = nc.dram_tensor("wpack", [P, len(K_CHUNKS), NSO], f32, kind="ExternalInput")
    ident_h = nc.dram_tensor("ident", [P, P], f32, kind="ExternalInput")
    gind_h = nc.dram_tensor("gind", [C_OUT, NGROUPS], f32, kind="ExternalInput")
    gexp_h = nc.dram_tensor("gexp", [NGROUPS, C_OUT], f32, kind="ExternalInput")
    gamma_h = nc.dram_tensor("gamma2", [C_OUT, 1], f32, kind="ExternalInput")
    beta_h = nc.dram_tensor("beta2", [C_OUT, 1], f32, kind="ExternalInput")
    svec_h = nc.dram_tensor("svec", [2 * C_OUT, 1], f32, kind="ExternalInput")
    out_h = nc.dram_tensor("out", [C_OUT, n_core], f32, kind="ExternalOutput")

    # ---- internal DRAM ----
    yscr_h = nc.dram_tensor("yscr", [C_OUT, n_pad], f32)
    if debug:
        dbgxg_h = nc.dram_tensor("dbgxg", [P, 128], f32, kind="ExternalOutput")
        dbgxp_h = nc.dram_tensor("dbgxp", [P, 512], f32, kind="ExternalOutput")
        dbgsel_h = nc.dram_tensor("dbgsel", [C_OUT, 512], f32, kind="ExternalOutput")
        dbgst_h = nc.dram_tensor("dbgst", [C_OUT, 8], f32, kind="ExternalOutput")
    cc_in_h = nc.dram_tensor("cc_in", [C_OUT, 2], f32)
    cc_out_h = nc.dram_tensor("cc_out", [C_OUT, 2], f32, addr_space="Shared")

    total_count = float(4 * n_total)  # values per group: 4 channels x N

    with tile.TileContext(nc) as tc, ExitStack() as outer:
        persist = outer.enter_context(tc.tile_pool(name="persist", bufs=1))
        gamma_t = persist.tile([C_OUT, 1], f32)
        nc.sync.dma_start(out=gamma_t[:], in_=gamma_h[:, :])
        beta_t = persist.tile([C_OUT, 1], f32)
        nc.sync.dma_start(out=beta_t[:], in_=beta_h[:, :])
        gind_t = persist.tile([C_OUT, NGROUPS], f32)
        nc.sync.dma_start(out=gind_t[:], in_=gind_h[:, :])
        gexp_t = persist.tile([NGROUPS, C_OUT], f32)
        nc.sync.dma_start(out=gexp_t[:], in_=gexp_h[:, :])
        statsS = persist.tile([C_OUT, NT], f32)
        nc.vector.memset(statsS[:], 0.0)
        statsQ = persist.tile([C_OUT, NT], f32)
        nc.vector.memset(statsQ[:], 0.0)

        # ================= phase 1 =================
        with ExitStack() as ph1:
            singles = ph1.enter_context(tc.tile_pool(name="singles", bufs=1))
            idxp = ph1.enter_context(tc.tile_pool(name="idxp", bufs=2))
            ldp = ph1.enter_context(tc.tile_pool(name="ldp", bufs=2))
            xgp = ph1.enter_context(tc.tile_pool(name="xgp", bufs=2))
            xpp = ph1.enter_context(tc.tile_pool(name="xpp", bufs=4))
            xpp3 = ph1.enter_context(tc.tile_pool(name="xpp3", bufs=1))
            selp = ph1.enter_context(tc.tile_pool(name="selp", bufs=2))
            sqp = ph1.enter_context(tc.tile_pool(name="sqp", bufs=2))
            tpsum = ph1.enter_context(tc.tile_pool(name="tpsum", bufs=3, space="PSUM"))
            apsum = ph1.enter_context(tc.tile_pool(name="apsum", bufs=2, space="PSUM"))

            wtile = singles.tile([P, len(K_CHUNKS), NSO], f32)
            nc.sync.dma_start(out=wtile[:], in_=w_h[:, :, :])
            ident_t = singles.tile([P, P], f32)
            nc.sync.dma_start(out=ident_t[:], in_=ident_h[:, :])
            svec_t = singles.tile([2 * C_OUT, 1], f32)
            nc.sync.dma_start(out=svec_t[:], in_=svec_h[:, :])

            gi = 0
            for base, nsub_s in sts:
                SUP_s = nsub_s * P

                idxt = idxp.tile([P, nsub_s, K], i32, tag="idxt")
                nc.sync.dma_start(
                    out=idxt[:],
                    in_=bass.AP(
                        tensor=nidx_h,
                        offset=base * K,
                        ap=[[K, P], [P * K, nsub_s], [1, K]],
                    ),
                )

                ldb = ldp.tile([2 * C_OUT, SUP_s], f32, tag="ldb")
                nc.gpsimd.dma_start(
                    out=ldb[:],
                    in_=bass.AP(
                        tensor=ld_h, offset=base, ap=[[0, 2 * C_OUT], [1, SUP_s]]
                    ),
                )
                m64 = ldp.tile([2 * C_OUT, SUP_s], i32, tag="m64")
                nc.vector.tensor_scalar(
                    out=m64[:],
                    in0=ldb[:],
                    scalar1=svec_t[:],
                    scalar2=None,
                    op0=mybir.AluOpType.is_equal,
                )

                selbuf = selp.tile([C_OUT, SUP_s], f32, tag="selbuf")

                xps = []
                for c, (k0, kc) in enumerate(K_CHUNKS):
                    F = kc * C_IN
                    xg = xgp.tile([P, nsub_s, F], f32, tag="xg")
                    for sub in range(nsub_s):
                        for b in range(kc):
                            nc.gpsimd.indirect_dma_start(
                                out=xg[:, sub, b * C_IN : (b + 1) * C_IN],
                                out_offset=None,
                                in_=xt_h[:, :],
                                in_offset=bass.IndirectOffsetOnAxis(
                                    ap=idxt[:, sub, k0 + b : k0 + b + 1], axis=0
                                ),
                            )
                    if kc == 8:
                        xp = xpp.tile([F, SUP_s], f32, tag="xp")
                    else:
                        xp = xpp3.tile([F, SUP_s], f32, tag="xp3")
                    for g0 in range(0, nsub_s, 4):
                        gs = min(4, nsub_s - g0)
                        tp4 = tpsum.tile([F, gs, P], f32, tag="tp4")
                        for j in range(gs):
                            nc.tensor.transpose(
                                out=tp4[:, j, :], in_=xg[:, g0 + j, :],
                                identity=ident_t[:],
                            )
                        nc.scalar.copy(
                            out=xp[:, g0 * P : (g0 + gs) * P], in_=tp4[:, :, :]
                        )
                    xps.append(xp)

                for t in range((SUP_s + mm - 1) // mm):
                    mm_t = min(mm, SUP_s - t * mm)
                    acc = apsum.tile([NSO, mm_t], f32, tag="acc")
                    for c, (k0, kc) in enumerate(K_CHUNKS):
                        F = kc * C_IN
                        lhsT = wtile[:F, c, :]
                        rhs = xps[c][:, t * mm : t * mm + mm_t]
                        if use_f32r:
                            lhsT = lhsT.bitcast(f32r)
                            rhs = rhs.bitcast(f32r)
                        nc.tensor.matmul(
                            out=acc[:], lhsT=lhsT, rhs=rhs,
                            start=(c == 0), stop=(c == len(K_CHUNKS) - 1),
                        )

                    sel = selbuf[:, t * mm : t * mm + mm_t]
                    nc.vector.tensor_copy(out=sel, in_=acc[0:C_OUT, :])
                    nc.vector.copy_predicated(
                        out=sel,
                        mask=m64[0:C_OUT, t * mm : t * mm + mm_t],
                        data=acc[C_OUT : 2 * C_OUT, :],
                    )
                    nc.vector.copy_predicated(
                        out=sel,
                        mask=m64[C_OUT : 2 * C_OUT, t * mm : t * mm + mm_t],
                        data=acc[2 * C_OUT : 3 * C_OUT, :],
                    )

                    v = min(mm_t, n_core - (base + t * mm))
                    if v > 0:
                        nc.vector.tensor_reduce(
                            out=statsS[:, gi : gi + 1],
                            in_=sel[:, :v],
                            axis=mybir.AxisListType.X,
                            op=mybir.AluOpType.add,
                        )
                        sq = sqp.tile([C_OUT, mm], f32, tag="sq")
                        nc.vector.scalar_tensor_tensor(
                            out=sq[:, :v],
                            in0=sel[:, :v],
                            scalar=0.0,
                            in1=sel[:, :v],
                            op0=mybir.AluOpType.add,
                            op1=mybir.AluOpType.mult,
                            accum_out=statsQ[:, gi : gi + 1],
                        )
                    gi += 1

                if debug and base == 0:
                    nc.sync.dma_start(out=dbgsel_h[:, :], in_=selbuf[:, 0:512])
                nc.sync.dma_start(
                    out=bass.AP(
                        tensor=yscr_h, offset=base, ap=[[n_pad, C_OUT], [1, SUP_s]]
                    ),
                    in_=selbuf[:],
                )

        # ================= phase 2: GN stats + AllReduce =================
        with ExitStack() as ph2s:
            ph2 = ph2s.enter_context(tc.tile_pool(name="ph2", bufs=1))
            p2sum = ph2s.enter_context(tc.tile_pool(name="p2sum", bufs=1, space="PSUM"))

            cct = ph2.tile([C_OUT, 2], f32)
            nc.vector.tensor_reduce(
                out=cct[:, 0:1],
                in_=statsS[:],
                axis=mybir.AxisListType.X,
                op=mybir.AluOpType.add,
            )
            nc.vector.tensor_reduce(
                out=cct[:, 1:2],
                in_=statsQ[:],
                axis=mybir.AxisListType.X,
                op=mybir.AluOpType.add,
            )
            nc.sync.dma_start(out=cc_in_h[:, :], in_=cct[:])
            nc.gpsimd.collective_compute(
                "AllReduce",
                mybir.AluOpType.add,
                replica_groups=[list(range(N_CORES))],
                ins=[cc_in_h.ap().opt()],
                outs=[cc_out_h.ap().opt()],
            )
            cc_sb = ph2.tile([C_OUT, 2], f32)
            nc.sync.dma_start(out=cc_sb[:], in_=cc_out_h[:, :])

            # group-combine: [32,2] -> [8,2] (sum 4 rows) -> expand back [32,2]
            ps8 = p2sum.tile([NGROUPS, 2], f32)
            nc.tensor.matmul(
                out=ps8[:], lhsT=gind_t[:], rhs=cc_sb[:], start=True, stop=True
            )
            gs8 = ph2.tile([NGROUPS, 2], f32)
            nc.vector.tensor_copy(out=gs8[:], in_=ps8[:])
            ps32 = p2sum.tile([C_OUT, 2], f32)
            nc.tensor.matmul(
                out=ps32[:], lhsT=gexp_t[:], rhs=gs8[:], start=True, stop=True
            )
            rs = ph2.tile([C_OUT, 2], f32)
            nc.vector.tensor_copy(out=rs[:], in_=ps32[:])

            mu = ph2.tile([C_OUT, 1], f32)
            nc.vector.tensor_scalar_mul(mu[:], rs[:, 0:1], 1.0 / total_count)
            e2 = ph2.tile([C_OUT, 1], f32)
            nc.vector.tensor_scalar_mul(e2[:], rs[:, 1:2], 1.0 / total_count)
            var = ph2.tile([C_OUT, 1], f32)
            nc.vector.scalar_tensor_tensor(
                out=var[:],
                in0=mu[:],
                scalar=-1.0,
                in1=mu[:],
                op0=mybir.AluOpType.mult,
                op1=mybir.AluOpType.mult,
            )
            nc.vector.tensor_add(var[:], var[:], e2[:])
            eps_t = ph2.tile([C_OUT, 1], f32)
            nc.vector.memset(eps_t[:], EPS)
            sd = ph2.tile([C_OUT, 1], f32)
            nc.scalar.activation(
                out=sd[:], in_=var[:], func=mybir.ActivationFunctionType.Sqrt,
                bias=eps_t[:],
            )
            rstd = ph2.tile([C_OUT, 1], f32)
            nc.vector.reciprocal(rstd[:], sd[:])
            scale_t = persist.tile([C_OUT, 1], f32)
            nc.vector.tensor_mul(scale_t[:], gamma_t[:], rstd[:])
            bias_t = persist.tile([C_OUT, 1], f32)
            nc.vector.scalar_tensor_tensor(
                out=bias_t[:],
                in0=mu[:],
                scalar=-1.0,
                in1=scale_t[:],
                op0=mybir.AluOpType.mult,
                op1=mybir.AluOpType.mult,
            )
            nc.vector.tensor_add(bias_t[:], bias_t[:], beta_t[:])
            if debug:
                dbg8 = ph2.tile([C_OUT, 8], f32)
                nc.vector.tensor_copy(out=dbg8[:, 0:2], in_=cct[:])
                nc.vector.tensor_copy(out=dbg8[:, 2:4], in_=cc_sb[:])
                nc.vector.tensor_copy(out=dbg8[:, 4:5], in_=mu[:])
                nc.vector.tensor_copy(out=dbg8[:, 5:6], in_=var[:])
                nc.vector.tensor_copy(out=dbg8[:, 6:7], in_=scale_t[:])
                nc.vector.tensor_copy(out=dbg8[:, 7:8], in_=bias_t[:])
                nc.sync.dma_start(out=dbgst_h[:, :], in_=dbg8[:])

        # ================= phase 3: normalize + relu =================
        with ExitStack() as ph3s:
            ph3 = ph3s.enter_context(tc.tile_pool(name="ph3", bufs=3))
            for off, nsub_s in sts:
                SUP_s = nsub_s * P
                v = min(SUP_s, n_core - off)
                yt = ph3.tile([C_OUT, SUP_s], f32, tag="yt")
                nc.sync.dma_start(
                    out=yt[:],
                    in_=bass.AP(
                        tensor=yscr_h, offset=off, ap=[[n_pad, C_OUT], [1, SUP_s]]
                    ),
                )
                ot = ph3.tile([C_OUT, SUP_s], f32, tag="ot")
                nc.scalar.activation(
                    out=ot[:],
                    in_=yt[:],
                    func=mybir.ActivationFunctionType.Relu,
                    bias=bias_t[:],
                    scale=scale_t[:],
                )
                nc.sync.dma_start(
                    out=bass.AP(
                        tensor=out_h, offset=off, ap=[[n_core, C_OUT], [1, v]]
                    ),
                    in_=ot[:, :v],
                )

    nc.finalize()
    return nc


def make_host_inputs(x, weight, gamma, beta, neighbor_idx, level_deltas,
                     nsub: int = 16):
    """Host-side prep: transpose x, pack weights, pad per-core index slices."""
    x = np.asarray(x, dtype=np.float32)
    weight = np.asarray(weight, dtype=np.float32)
    gamma = np.asarray(gamma, dtype=np.float32)
    beta = np.asarray(beta, dtype=np.float32)
    neighbor_idx = np.asarray(neighbor_idx, dtype=np.int32)
    level_deltas = np.asarray(level_deltas, dtype=np.int32)

    n_total = x.shape[2]
    n_core = n_total // N_CORES
    _, n_pad = geometry(n_core, nsub)

    xt = np.ascontiguousarray(x[0].T)  # [N, 16]

    wpack = np.zeros((P, len(K_CHUNKS), NSO), dtype=np.float32)
    for c, (k0, kc) in enumerate(K_CHUNKS):
        wc = weight[:, :, :, k0 : k0 + kc]  # [S, O, C, kc]
        wpack[: kc * C_IN, c, :] = wc.transpose(3, 2, 0, 1).reshape(kc * C_IN, NSO)

    ident = np.eye(P, dtype=np.float32)
    gind = np.zeros((C_OUT, NGROUPS), dtype=np.float32)
    for r in range(C_OUT):
        gind[r, r // (C_OUT // NGROUPS)] = 1.0
    gexp = np.ascontiguousarray(gind.T)
    svec = np.concatenate(
        [np.full(C_OUT, 1.0, np.float32), np.full(C_OUT, 2.0, np.float32)]
    ).reshape(2 * C_OUT, 1)

    shared = {
        "xt": xt,
        "wpack": wpack,
        "ident": ident,
        "gind": gind,
        "gexp": gexp,
        "gamma2": gamma.reshape(C_OUT, 1),
        "beta2": beta.reshape(C_OUT, 1),
        "svec": svec,
    }

    in_maps = []
    for c in range(N_CORES):
        lo, hi = c * n_core, (c + 1) * n_core
        nidx_c = np.zeros((n_pad, K), dtype=np.int32)
        nidx_c[:n_core] = neighbor_idx[lo:hi]
        ld_c = np.zeros((n_pad,), dtype=np.float32)
        ld_c[:n_core] = level_deltas[lo:hi].astype(np.float32)
        in_maps.append({**shared, "nidx": nidx_c, "ld": ld_c})
    return in_maps


_NC_CACHE: dict = {}


def kernel(x, weight, gamma, beta, neighbor_idx, level_deltas,
           nsub: int = 16, mm: int = 512, use_f32r: bool = False,
           _run_kwargs: dict | None = None):
    x = np.asarray(x)
    n_total = x.shape[2]
    key = (n_total, nsub, mm, use_f32r)
    if key not in _NC_CACHE:
        _NC_CACHE[key] = build_nc(n_total, nsub=nsub, mm=mm, use_f32r=use_f32r)
    nc = _NC_CACHE[key]

    in_maps = make_host_inputs(
        x, weight, gamma, beta, neighbor_idx, level_deltas, nsub=nsub
    )
    res = run_bass_kernel_spmd(
        nc, in_maps, list(range(N_CORES)), **(_run_kwargs or {})
    )
    n_core = n_total // N_CORES
    parts = [np.asarray(res.results[c]["out"]) for c in range(N_CORES)]
    full = np.concatenate(parts, axis=1).reshape(1, C_OUT, n_total)
    if _run_kwargs:
        kernel.last_results = res
    return full.astype(np.float32)



# revision 3
# speedup vs baseline: 26.2889x; 26.2889x over previous
"""APR conv (gather + per-particle stencil select) + GroupNorm + ReLU on 8 trn2 cores.

Sharding: particles (last axis N) split across 8 cores; the gather source
x^T [N, 16] f32 is replicated on every core; stencil weights / GN params
replicated; GroupNorm reduction over N becomes an AllReduce of per-group
(sum, sumsq).

Per-core pipeline (phase 1, per 2048-particle supertile):
  - one strided DMA loads neighbor_idx block [128, 16, 27] i32
  - 4 indirect DMA gathers (k-chunks of 8,8,8,3) pull 64B feature rows from
    x^T in DRAM -> SBUF [128p, nsub, kc*16]
  - PE transposes [128,128] blocks so (k, c) lands on partitions
  - PSUM-accumulated matmuls vs packed weights [(k c), (s o)] -> acc [96, 512]
  - DVE copy_predicated selects the stencil per particle -> sel [32, 512]
  - per-tile sum / sumsq accumulate for GroupNorm; sel written to DRAM scratch
Phase 2: per-core (sum, sumsq) [32, 2] -> AllReduce(add) over 8 cores ->
  scale/bias [32, 1] per channel.
Phase 3: read scratch back, one ACT op per tile: Relu(y * scale + bias).
"""

import math
from contextlib import ExitStack

import numpy as np

import concourse.bacc as bacc
import concourse.bass as bass
import concourse.tile as tile
from concourse import mybir
from concourse.bass_utils import run_bass_kernel_spmd

N_CORES = 8
C_IN = 16
C_OUT = 32
NS = 3  # stencils
K = 27
NGROUPS = 8
EPS = 1e-5
P = 128

K_CHUNKS = [(0, 8), (8, 8), (16, 8), (24, 3)]
NSO = NS * C_OUT  # 96

f32 = mybir.dt.float32
f32r = mybir.dt.float32r
i32 = mybir.dt.int32


def geometry(n_core: int, nsub: int):
    """Supertile list [(base, nsub_s)] covering n_core, padded per tile."""
    tiles = []
    base = 0
    while base < n_core:
        rem = n_core - base
        ns = nsub if rem >= P * nsub else (rem + P - 1) // P
        tiles.append((base, ns))
        base += ns * P
    return tiles, base


def build_nc(n_total: int, nsub: int = 16, mm: int = 512, use_f32r: bool = False, debug: bool = False):
    """Build the per-core Bass program (SPMD; same program on all cores)."""
    assert n_total % N_CORES == 0
    n_core = n_total // N_CORES
    SUP = P * nsub  # particles per (full) supertile
    assert SUP % mm == 0 and mm % P == 0
    sts, n_pad = geometry(n_core, nsub)
    n_sup = len(sts)
    NT = sum((ns * P + mm - 1) // mm for _, ns in sts)  # stat slots

    nc = bacc.Bacc(
        "TRN2",
        target_bir_lowering=False,
        debug=False,
        num_devices=N_CORES,
    )

    # ---- I/O ----
    xt_h = nc.dram_tensor("xt", [n_total, C_IN], f32, kind="ExternalInput")
    nidx_h = nc.dram_tensor("nidx", [n_pad, K], i32, kind="ExternalInput")
    ld_h = nc.dram_tensor("ld", [n_pad], f32, kind="ExternalInput")
    w_h = nc.dram_tensor("wpack", [P, len(K_CHUNKS), NSO], f32, kind="ExternalInput")
    ident_h = nc.dram_tensor("ident", [P, P], f32, kind="ExternalInput")
    gind_h = nc.dram_tensor("gind", [C_OUT, NGROUPS], f32, kind="ExternalInput")
    gexp_h = nc.dram_tensor("gexp", [NGROUPS, C_OUT], f32, kind="ExternalInput")
    gamma_h = nc.dram_tensor("gamma2", [C_OUT, 1], f32, kind="ExternalInput")
    beta_h = nc.dram_tensor("beta2", [C_OUT, 1], f32, kind="ExternalInput")
    svec_h = nc.dram_tensor("svec", [2 * C_OUT, 1], f32, kind="ExternalInput")
    out_h = nc.dram_tensor("out", [C_OUT, n_core], f32, kind="ExternalOutput")

    # ---- internal DRAM ----
    yscr_h = nc.dram_tensor("yscr", [C_OUT, n_pad], f32)
    if debug:
        dbgxg_h = nc.dram_tensor("dbgxg", [P, 128], f32, kind="ExternalOutput")
        dbgxp_h = nc.dram_tensor("dbgxp", [P, 512], f32, kind="ExternalOutput")
        dbgsel_h = nc.dram_tensor("dbgsel", [C_OUT, 512], f32, kind="ExternalOutput")
        dbgst_h = nc.dram_tensor("dbgst", [C_OUT, 8], f32, kind="ExternalOutput")
    cc_in_h = nc.dram_tensor("cc_in", [C_OUT, 2], f32)
    cc_out_h = nc.dram_tensor("cc_out", [C_OUT, 2], f32, addr_space="Shared")

    total_count = float(4 * n_total)  # values per group: 4 channels x N

    with tile.TileContext(nc) as tc, ExitStack() as outer:
        persist = outer.enter_context(tc.tile_pool(name="persist", bufs=1))
        gamma_t = persist.tile([C_OUT, 1], f32)
        nc.sync.dma_start(out=gamma_t[:], in_=gamma_h[:, :])
        beta_t = persist.tile([C_OUT, 1], f32)
        nc.sync.dma_start(out=beta_t[:], in_=beta_h[:, :])
        gind_t = persist.tile([C_OUT, NGROUPS], f32)
        nc.sync.dma_start(out=gind_t[:], in_=gind_h[:, :])
        gexp_t = persist.tile([NGROUPS, C_OUT], f32)
        nc.sync.dma_start(out=gexp_t[:], in_=gexp_h[:, :])
        statsS = persist.tile([C_OUT, NT], f32)
        nc.vector.memset(statsS[:], 0.0)
        statsQ = persist.tile([C_OUT, NT], f32)
        nc.vector.memset(statsQ[:], 0.0)

        # ================= phase 1 =================
        with ExitStack() as ph1:
            singles = ph1.enter_context(tc.tile_pool(name="singles", bufs=1))
            idxp = ph1.enter_context(tc.tile_pool(name="idxp", bufs=2))
            ldp = ph1.enter_context(tc.tile_pool(name="ldp", bufs=2))
            xgp = ph1.enter_context(tc.tile_pool(name="xgp", bufs=2))
            xpp = ph1.enter_context(tc.tile_pool(name="xpp", bufs=4))
            xpp3 = ph1.enter_context(tc.tile_pool(name="xpp3", bufs=1))
            selp = ph1.enter_context(tc.tile_pool(name="selp", bufs=2))
            sqp = ph1.enter_context(tc.tile_pool(name="sqp", bufs=2))
            tpsum = ph1.enter_context(tc.tile_pool(name="tpsum", bufs=3, space="PSUM"))
            apsum = ph1.enter_context(tc.tile_pool(name="apsum", bufs=2, space="PSUM"))

            wtile = singles.tile([P, len(K_CHUNKS), NSO], f32)
            nc.sync.dma_start(out=wtile[:], in_=w_h[:, :, :])
            ident_t = singles.tile([P, P], f32)
            nc.sync.dma_start(out=ident_t[:], in_=ident_h[:, :])
            svec_t = singles.tile([2 * C_OUT, 1], f32)
            nc.sync.dma_start(out=svec_t[:], in_=svec_h[:, :])

            gi = 0
            for base, nsub_s in sts:
                SUP_s = nsub_s * P

                idxt = idxp.tile([P, nsub_s, K], i32, tag="idxt")
                nc.sync.dma_start(
                    out=idxt[:],
                    in_=bass.AP(
                        tensor=nidx_h,
                        offset=base * K,
                        ap=[[K, P], [P * K, nsub_s], [1, K]],
                    ),
                )

                ldb = ldp.tile([2 * C_OUT, SUP_s], f32, tag="ldb")
                nc.gpsimd.dma_start(
                    out=ldb[:],
                    in_=bass.AP(
                        tensor=ld_h, offset=base, ap=[[0, 2 * C_OUT], [1, SUP_s]]
                    ),
                )
                m64 = ldp.tile([2 * C_OUT, SUP_s], i32, tag="m64")
                nc.vector.tensor_scalar(
                    out=m64[:],
                    in0=ldb[:],
                    scalar1=svec_t[:],
                    scalar2=None,
                    op0=mybir.AluOpType.is_equal,
                )

                selbuf = selp.tile([C_OUT, SUP_s], f32, tag="selbuf")

                xps = []
                for c, (k0, kc) in enumerate(K_CHUNKS):
                    F = kc * C_IN
                    xg = xgp.tile([P, nsub_s, F], f32, tag="xg")
                    for sub in range(nsub_s):
                        for b in range(kc):
                            nc.gpsimd.indirect_dma_start(
                                out=xg[:, sub, b * C_IN : (b + 1) * C_IN],
                                out_offset=None,
                                in_=xt_h[:, :],
                                in_offset=bass.IndirectOffsetOnAxis(
                                    ap=idxt[:, sub, k0 + b : k0 + b + 1], axis=0
                                ),
                            )
                    if kc == 8:
                        xp = xpp.tile([F, SUP_s], f32, tag="xp")
                    else:
                        xp = xpp3.tile([F, SUP_s], f32, tag="xp3")
                    for g0 in range(0, nsub_s, 4):
                        gs = min(4, nsub_s - g0)
                        tp4 = tpsum.tile([F, gs, P], f32, tag="tp4")
                        for j in range(gs):
                            nc.tensor.transpose(
                                out=tp4[:, j, :], in_=xg[:, g0 + j, :],
                                identity=ident_t[:],
                            )
                        nc.scalar.copy(
                            out=xp[:, g0 * P : (g0 + gs) * P], in_=tp4[:, :, :]
                        )
                    xps.append(xp)

                for t in range((SUP_s + mm - 1) // mm):
                    mm_t = min(mm, SUP_s - t * mm)
                    acc = apsum.tile([NSO, mm_t], f32, tag="acc")
                    for c, (k0, kc) in enumerate(K_CHUNKS):
                        F = kc * C_IN
                        lhsT = wtile[:F, c, :]
                        rhs = xps[c][:, t * mm : t * mm + mm_t]
                        if use_f32r:
                            lhsT = lhsT.bitcast(f32r)
                            rhs = rhs.bitcast(f32r)
                        nc.tensor.matmul(
                            out=acc[:], lhsT=lhsT, rhs=rhs,
                            start=(c == 0), stop=(c == len(K_CHUNKS) - 1),
                        )

                    sel = selbuf[:, t * mm : t * mm + mm_t]
                    nc.vector.tensor_copy(out=sel, in_=acc[0:C_OUT, :])
                    nc.vector.copy_predicated(
                        out=sel,
                        mask=m64[0:C_OUT, t * mm : t * mm + mm_t],
                        data=acc[C_OUT : 2 * C_OUT, :],
                    )
                    nc.vector.copy_predicated(
                        out=sel,
                        mask=m64[C_OUT : 2 * C_OUT, t * mm : t * mm + mm_t],
                        data=acc[2 * C_OUT : 3 * C_OUT, :],
                    )

                    v = min(mm_t, n_core - (base + t * mm))
                    if v > 0:
                        nc.vector.tensor_reduce(
                            out=statsS[:, gi : gi + 1],
                            in_=sel[:, :v],
                            axis=mybir.AxisListType.X,
                            op=mybir.AluOpType.add,
                        )
                        sq = sqp.tile([C_OUT, mm], f32, tag="sq")
                        nc.vector.scalar_tensor_tensor(
                            out=sq[:, :v],
                            in0=sel[:, :v],
                            scalar=0.0,
                            in1=sel[:, :v],
                            op0=mybir.AluOpType.add,
                            op1=mybir.AluOpType.mult,
                            accum_out=statsQ[:, gi : gi + 1],
                        )
                    gi += 1

                if debug and base == 0:
                    nc.sync.dma_start(out=dbgsel_h[:, :], in_=selbuf[:, 0:512])
                nc.sync.dma_start(
                    out=bass.AP(
                        tensor=yscr_h, offset=base, ap=[[n_pad, C_OUT], [1, SUP_s]]
                    ),
                    in_=selbuf[:],
                )

        # ================= phase 2: GN stats + AllReduce =================
        with ExitStack() as ph2s:
            ph2 = ph2s.enter_context(tc.tile_pool(name="ph2", bufs=1))
            p2sum = ph2s.enter_context(tc.tile_pool(name="p2sum", bufs=1, space="PSUM"))

            cct = ph2.tile([C_OUT, 2], f32)
            nc.vector.tensor_reduce(
                out=cct[:, 0:1],
                in_=statsS[:],
                axis=mybir.AxisListType.X,
                op=mybir.AluOpType.add,
            )
            nc.vector.tensor_reduce(
                out=cct[:, 1:2],
                in_=statsQ[:],
                axis=mybir.AxisListType.X,
                op=mybir.AluOpType.add,
            )
            nc.sync.dma_start(out=cc_in_h[:, :], in_=cct[:])
            nc.gpsimd.collective_compute(
                "AllReduce",
                mybir.AluOpType.add,
                replica_groups=[list(range(N_CORES))],
                ins=[cc_in_h.ap().opt()],
                outs=[cc_out_h.ap().opt()],
            )
            cc_sb = ph2.tile([C_OUT, 2], f32)
            nc.sync.dma_start(out=cc_sb[:], in_=cc_out_h[:, :])

            # group-combine: [32,2] -> [8,2] (sum 4 rows) -> expand back [32,2]
            ps8 = p2sum.tile([NGROUPS, 2], f32)
            nc.tensor.matmul(
                out=ps8[:], lhsT=gind_t[:], rhs=cc_sb[:], start=True, stop=True
            )
            gs8 = ph2.tile([NGROUPS, 2], f32)
            nc.vector.tensor_copy(out=gs8[:], in_=ps8[:])
            ps32 = p2sum.tile([C_OUT, 2], f32)
            nc.tensor.matmul(
                out=ps32[:], lhsT=gexp_t[:], rhs=gs8[:], start=True, stop=True
            )
            rs = ph2.tile([C_OUT, 2], f32)
            nc.vector.tensor_copy(out=rs[:], in_=ps32[:])

            mu = ph2.tile([C_OUT, 1], f32)
            nc.vector.tensor_scalar_mul(mu[:], rs[:, 0:1], 1.0 / total_count)
            e2 = ph2.tile([C_OUT, 1], f32)
            nc.vector.tensor_scalar_mul(e2[:], rs[:, 1:2], 1.0 / total_count)
            var = ph2.tile([C_OUT, 1], f32)
            nc.vector.scalar_tensor_tensor(
                out=var[:],
                in0=mu[:],
                scalar=-1.0,
                in1=mu[:],
                op0=mybir.AluOpType.mult,
                op1=mybir.AluOpType.mult,
            )
            nc.vector.tensor_add(var[:], var[:], e2[:])
            eps_t = ph2.tile([C_OUT, 1], f32)
            nc.vector.memset(eps_t[:], EPS)
            sd = ph2.tile([C_OUT, 1], f32)
            nc.scalar.activation(
                out=sd[:], in_=var[:], func=mybir.ActivationFunctionType.Sqrt,
                bias=eps_t[:],
            )
            rstd = ph2.tile([C_OUT, 1], f32)
            nc.vector.reciprocal(rstd[:], sd[:])
            scale_t = persist.tile([C_OUT, 1], f32)
            nc.vector.tensor_mul(scale_t[:], gamma_t[:], rstd[:])
            bias_t = persist.tile([C_OUT, 1], f32)
            nc.vector.scalar_tensor_tensor(
                out=bias_t[:],
                in0=mu[:],
                scalar=-1.0,
                in1=scale_t[:],
                op0=mybir.AluOpType.mult,
                op1=mybir.AluOpType.mult,
            )
            nc.vector.tensor_add(bias_t[:], bias_t[:], beta_t[:])
            if debug:
                dbg8 = ph2.tile([C_OUT, 8], f32)
                nc.vector.tensor_copy(out=dbg8[:, 0:2], in_=cct[:])
                nc.vector.tensor_copy(out=dbg8[:, 2:4], in_=cc_sb[:])
                nc.vector.tensor_copy(out=dbg8[:, 4:5], in_=mu[:])
                nc.vector.tensor_copy(out=dbg8[:, 5:6], in_=var[:])
                nc.vector.tensor_copy(out=dbg8[:, 6:7], in_=scale_t[:])
                nc.vector.tensor_copy(out=dbg8[:, 7:8], in_=bias_t[:])
                nc.sync.dma_start(out=dbgst_h[:, :], in_=dbg8[:])

        # ================= phase 3: normalize + relu =================
        with ExitStack() as ph3s:
            ph3 = ph3s.enter_context(tc.tile_pool(name="ph3", bufs=3))
            for off, nsub_s in sts:
                SUP_s = nsub_s * P
                v = min(SUP_s, n_core - off)
                yt = ph3.tile([C_OUT, SUP_s], f32, tag="yt")
                nc.sync.dma_start(
                    out=yt[:],
                    in_=bass.AP(
                        tensor=yscr_h, offset=off, ap=[[n_pad, C_OUT], [1, SUP_s]]
                    ),
                )
                ot = ph3.tile([C_OUT, SUP_s], f32, tag="ot")
                nc.scalar.activation(
                    out=ot[:],
                    in_=yt[:],
                    func=mybir.ActivationFunctionType.Relu,
                    bias=bias_t[:],
                    scale=scale_t[:],
                )
                nc.sync.dma_start(
                    out=bass.AP(
                        tensor=out_h, offset=off, ap=[[n_core, C_OUT], [1, v]]
                    ),
                    in_=ot[:, :v],
                )

    nc.finalize()
    return nc


def make_host_inputs(x, weight, gamma, beta, neighbor_idx, level_deltas,
                     nsub: int = 16):
    """Host-side prep: transpose x, pack weights, pad per-core index slices."""
    x = np.asarray(x, dtype=np.float32)
    weight = np.asarray(weight, dtype=np.float32)
    gamma = np.asarray(gamma, dtype=np.float32)
    beta = np.asarray(beta, dtype=np.float32)
    neighbor_idx = np.asarray(neighbor_idx, dtype=np.int32)
    level_deltas = np.asarray(level_deltas, dtype=np.int32)

    n_total = x.shape[2]
    n_core = n_total // N_CORES
    _, n_pad = geometry(n_core, nsub)

    xt = np.ascontiguousarray(x[0].T)  # [N, 16]

    wpack = np.zeros((P, len(K_CHUNKS), NSO), dtype=np.float32)
    for c, (k0, kc) in enumerate(K_CHUNKS):
        wc = weight[:, :, :, k0 : k0 + kc]  # [S, O, C, kc]
        wpack[: kc * C_IN, c, :] = wc.transpose(3, 2, 0, 1).reshape(kc * C_IN, NSO)

    ident = np.eye(P, dtype=np.float32)
    gind = np.zeros((C_OUT, NGROUPS), dtype=np.float32)
    for r in range(C_OUT):
        gind[r, r // (C_OUT // NGROUPS)] = 1.0
    gexp = np.ascontiguousarray(gind.T)
    svec = np.concatenate(
        [np.full(C_OUT, 1.0, np.float32), np.full(C_OUT, 2.0, np.float32)]
    ).reshape(2 * C_OUT, 1)

    shared = {
        "xt": xt,
        "wpack": wpack,
        "ident": ident,
        "gind": gind,
        "gexp": gexp,
        "gamma2": gamma.reshape(C_OUT, 1),
        "beta2": beta.reshape(C_OUT, 1),
        "svec": svec,
    }

    in_maps = []
    for c in range(N_CORES):
        lo, hi = c * n_core, (c + 1) * n_core
        nidx_c = np.zeros((n_pad, K), dtype=np.int32)
        nidx_c[:n_core] = neighbor_idx[lo:hi]
        ld_c = np.zeros((n_pad,), dtype=np.float32)
        ld_c[:n_core] = level_deltas[lo:hi].astype(np.float32)
        in_maps.append({**shared, "nidx": nidx_c, "ld": ld_c})
    return in_maps


_NC_CACHE: dict = {}


def kernel(x, weight, gamma, beta, neighbor_idx, level_deltas,
           nsub: int = 16, mm: int = 512, use_f32r: bool = False,
           _run_kwargs: dict | None = None):
    x = np.asarray(x)
    n_total = x.shape[2]
    key = (n_total, nsub, mm, use_f32r)
    if key not in _NC_CACHE:
        _NC_CACHE[key] = build_nc(n_total, nsub=nsub, mm=mm, use_f32r=use_f32r)
    nc = _NC_CACHE[key]

    in_maps = make_host_inputs(
        x, weight, gamma, beta, neighbor_idx, level_deltas, nsub=nsub
    )
    res = run_bass_kernel_spmd(
        nc, in_maps, list(range(N_CORES)), **(_run_kwargs or {})
    )
    n_core = n_total // N_CORES
    parts = [np.asarray(res.results[c]["out"]) for c in range(N_CORES)]
    full = np.concatenate(parts, axis=1).reshape(1, C_OUT, n_total)
    if _run_kwargs:
        kernel.last_results = res
    return full.astype(np.float32)

